# revision 15
# baseline (speedup 1.0000x reference)
"""Causal self-MQA kernel for Trainium2, sharded over 8 NeuronCores.

Problem: B=2, S=2048, D=2048, H=16 query heads, DH=128, single KV head,
GPT-NeoX RoPE, causal attention, fused q/kv/o projections.

Sharding: 8 cores = 2 batches x 4 head-groups (4 heads = 512 q-dims per
core). The tiny kv projection is replicated within a batch. Each core
computes a partial output [S, D] (its head-group's contribution through
the o-projection); the host sums the 4 partials per batch and adds
o_bias.

Datapath: fp16 activations end-to-end (PE fp16 = 1 col/cycle at any
width, so no f32r >=256 free-dim constraint and half the DMA bytes).
The two big weight GEMMs (q/kv projections, o-projection) run as
residual-fp8 DoubleRow matmuls: both operands are split hi+lo in
float8e4 with power-of-2 scaling (x*16, W*1024, attn*32), and the
product is computed as hi@hi + hi@lo + lo@hi at 0.5 cycles/col --
2x the fp16 rate with ~7-bit effective mantissa. The single shared
scale per term group means all three terms accumulate into one PSUM
bank; the descale folds into the (anyway needed) ACT/DVE copy out of
PSUM.

Layouts keep the feature dim on partitions so no activation transpose
is needed except the tiny V re-layout (32 PE transposes / core):
  qT[dh, s] = wqT.T @ xT          (DoubleRow pairs of d-tiles)
  rotate_half(q) = swap_matrix @ qT   (PE matmul; sign folded into sinT)
  scoresT[k, q] = k_ropeT(dh,k).T @ q_ropeT(dh,q)   (fp16, exact causal
    widths 512/384/256/128 -- no diagonal widening needed)
  softmax over k = PARTITION dim: no max-subtraction (scores < ~5),
    sums via ones-vector fp16 matmuls into a persistent 2-row PSUM tile
    (rows at partitions 0/32, alternating per unit, so the reciprocal
    read never serializes back-to-back units), reciprocal on DVE,
    partition-broadcast on GpSimd.
  attnT[dh, q] += v_nat(k,dh).T @ expT(k,q)   accumulated over k blocks
  attn stored directly as scaled fp8 hi/lo (32x) for the o-projection
  out_part[s_blk, d] = attn8 packs @ wo8 packs (DoubleRow), fp16 out
"""

import os
import sys

import numpy as np

for _p in ("/opt/trn_rl_repo", "/root/.axon_site/_ro/trn_rl_repo"):
    if os.path.isdir(_p) and _p not in sys.path:
        sys.path.insert(0, _p)

import ml_dtypes  # noqa: E402

import concourse.bass as bass  # noqa: E402,F401
import concourse.mybir as mybir  # noqa: E402
import concourse.tile as tile  # noqa: E402
from concourse import bacc  # noqa: E402
from concourse.bass_utils import run_bass_kernel_spmd  # noqa: E402

B, S, D = 2, 2048, 2048
H, DH = 16, 128
G = 4          # head groups (cores per batch)
HPG = 4        # heads per group
C = HPG * DH   # 512 output dims per group
SC = 256       # projection s-chunk width
NSC = S // SC  # 8
KT = D // 128  # 16 contraction tiles
KP = KT // 2   # 8 DoubleRow tile-pairs
QC = 512       # attention q-chunk width
NQC = S // QC  # 4
NSB = S // 128  # 16 s-blocks

SX = 16.0      # x fp8 scale
SW = 1024.0    # weight fp8 scale
SA = 32.0      # attn fp8 scale
PSC = 1.0 / (SX * SW)   # projection descale
OSC = 1.0 / (SA * SW)   # o-projection descale

F32 = mybir.dt.float32
F16 = mybir.dt.float16
F8 = mybir.dt.float8e4
AF = mybir.ActivationFunctionType
OP = mybir.AluOpType
DR = mybir.MatmulPerfMode.DoubleRow

NP_F8 = ml_dtypes.float8_e4m3

# packed fp16 consts layout: cost/sint interleaved per s-chunk, then
# ident / swap / onesk columns
C16_IDENT = NSC * 2 * SC            # 4096
C16_SWAP = C16_IDENT + 128          # 4224
C16_ONES = C16_SWAP + 128           # 4352
C16_W = C16_ONES + 1                # 4353
# packed f32 consts: mask, then qb (HPG cols), then kvb (2 cols)
C32_QB = 128
C32_KVB = C32_QB + HPG              # 132
C32_W = C32_KVB + 2                 # 134

_NC_CACHE = {}


def build_nc():
    nc = bacc.Bacc("TRN2", target_bir_lowering=False, debug=False)

    # all inputs pre-packed on the host into exact SBUF consumption layout
    # so every DMA moves >=2KB contiguous runs (no <512B descriptor penalty)
    xh = nc.dram_tensor("xh", [128, NSC, KT, SC], F8, kind="ExternalInput").ap()
    xl = nc.dram_tensor("xl", [128, NSC, KT, SC], F8, kind="ExternalInput").ap()
    wqh = nc.dram_tensor("wqh", [128, HPG, KT, DH], F8, kind="ExternalInput").ap()
    wql = nc.dram_tensor("wql", [128, HPG, KT, DH], F8, kind="ExternalInput").ap()
    wkvh = nc.dram_tensor("wkvh", [128, 2, KT, DH], F8, kind="ExternalInput").ap()
    wkvl = nc.dram_tensor("wkvl", [128, 2, KT, DH], F8, kind="ExternalInput").ap()
    woh = nc.dram_tensor("woh", [128, HPG, D], F8, kind="ExternalInput").ap()
    wol = nc.dram_tensor("wol", [128, HPG, D], F8, kind="ExternalInput").ap()
    c16 = nc.dram_tensor("c16", [128, C16_W], F16, kind="ExternalInput").ap()
    c32 = nc.dram_tensor("c32", [128, C32_W], F32, kind="ExternalInput").ap()
    out_p = nc.dram_tensor("out_p", [S, D], F16, kind="ExternalOutput").ap()

    with tile.TileContext(nc) as tc:
        _body(nc, tc, xh, xl, wqh, wql, wkvh, wkvl, woh, wol, c16, c32,
              out_p)
    nc.compile()
    return nc


def _body(nc, tc, xh, xl, wqh, wql, wkvh, wkvl, woh, wol, c16, c32, out_p):
    consts = tc.alloc_tile_pool(name="consts", bufs=1)
    sb = tc.alloc_tile_pool(name="sb", bufs=2)
    psum = tc.alloc_tile_pool(name="psum", bufs=1, space="PSUM")

    # ---- packed constants ----
    c16_sb = consts.tile([128, C16_W], F16, tag="c16", name="c16")
    c32_sb = consts.tile([128, C32_W], F32, tag="c32", name="c32")

    def cost_ap(ssl):  # ssl must be an s-chunk slice
        sc = ssl.start // SC
        off = sc * 2 * SC
        return c16_sb[:, off + (ssl.start - sc * SC):off + (ssl.stop - sc * SC)]

    def sint_ap(ssl):
        sc = ssl.start // SC
        off = sc * 2 * SC + SC
        return c16_sb[:, off + (ssl.start - sc * SC):off + (ssl.stop - sc * SC)]

    ident_sb = c16_sb[:, C16_IDENT:C16_IDENT + 128]
    swap_sb = c16_sb[:, C16_SWAP:C16_SWAP + 128]
    onesk_sb = c16_sb[:, C16_ONES:C16_ONES + 1]
    mask_sb = c32_sb[:, 0:128]
    qb_sb = c32_sb[:, C32_QB:C32_QB + HPG]
    kvb_sb = c32_sb[:, C32_KVB:C32_KVB + 2]

    # ---- weights (fp8 hi/lo) ----
    wqh_sb = consts.tile([128, HPG, KT, DH], F8, tag="wqh", name="wqh")
    wql_sb = consts.tile([128, HPG, KT, DH], F8, tag="wql", name="wql")
    wkvh_sb = consts.tile([128, 2, KT, DH], F8, tag="wkvh", name="wkvh")
    wkvl_sb = consts.tile([128, 2, KT, DH], F8, tag="wkvl", name="wkvl")

    # ---- persistent activations ----
    q_rope = [consts.tile([DH, S], F16, tag=f"qrope{h}", name=f"qrope{h}")
              for h in range(HPG)]
    k_rope = consts.tile([DH, S], F16, tag="krope", name="krope")
    v_nat = consts.tile([128, NSB, DH], F16, tag="vnat", name="vnat")
    attn8h = consts.tile([128, HPG, S], F8, tag="attn8h", name="attn8h")
    attn8l = consts.tile([128, HPG, S], F8, tag="attn8l", name="attn8l")

    # persistent 2-row softmax-sum PSUM tile (rows at partitions 0 and 32
    # so tile_position stays 32-aligned); alternating rows give the
    # reciprocal read two units of slack before the row is reused.
    sums_ps = psum.tile([64, QC], F32, tag="sums", name="sums")

    # ================= phase 1: q/kv projections + RoPE =================
    def alloc_x():
        return (sb.tile([128, KT, SC], F8, tag="xth", name="xth", bufs=3),
                sb.tile([128, KT, SC], F8, tag="xtl", name="xtl", bufs=3))

    xt_next = alloc_x()
    for sc in range(NSC):
        ssl = slice(sc * SC, (sc + 1) * SC)
        xth, xtl = xt_next
        if sc == 0:
            # startup: every input DMA on the (compute-free) SP queue, in
            # exact first-consumption order with small leading pieces. DMA
            # issues on the ACT/Pool queues would delay their compute.
            nc.sync.dma_start(wkvh_sb[:, 1, 0:2, :], wkvh[:, 1, 0:2, :])
            nc.sync.dma_start(xth[:, 0:8, :], xh[:, 0, 0:8, :])
            nc.sync.dma_start(wkvh_sb[:, 1, 2:KT, :], wkvh[:, 1, 2:KT, :])
            nc.sync.dma_start(xth[:, 8:16, :], xh[:, 0, 8:16, :])
            nc.sync.dma_start(xtl, xl[:, 0])
            nc.sync.dma_start(wkvl_sb[:, 1], wkvl[:, 1])    # v lo
            nc.sync.dma_start(wkvh_sb[:, 0], wkvh[:, 0])    # k hi
            nc.sync.dma_start(c32_sb, c32)
            nc.sync.dma_start(wkvl_sb[:, 0], wkvl[:, 0])    # k lo
            nc.sync.dma_start(c16_sb[:, C16_IDENT:C16_W],
                              c16[:, C16_IDENT:C16_W])
            nc.sync.dma_start(wqh_sb[:, 0], wqh[:, 0])
            nc.sync.dma_start(wql_sb[:, 0], wql[:, 0])
            nc.sync.dma_start(c16_sb[:, 0:1024], c16[:, 0:1024])
            nc.sync.dma_start(wqh_sb[:, 1], wqh[:, 1])
            nc.sync.dma_start(wql_sb[:, 1], wql[:, 1])
            xt_next = alloc_x()
            nc.sync.dma_start(xt_next[0], xh[:, 1])
            nc.sync.dma_start(wqh_sb[:, 2], wqh[:, 2])
            nc.sync.dma_start(wql_sb[:, 2], wql[:, 2])
            nc.sync.dma_start(xt_next[1], xl[:, 1])
            nc.sync.dma_start(wqh_sb[:, 3], wqh[:, 3])
            nc.sync.dma_start(wql_sb[:, 3], wql[:, 3])
            nc.sync.dma_start(c16_sb[:, 1024:C16_IDENT],
                              c16[:, 1024:C16_IDENT])
        elif sc < NSC - 1:
            xt_next = alloc_x()
            nc.sync.dma_start(xt_next[0], xh[:, sc + 1])
            nc.sync.dma_start(xt_next[1], xl[:, sc + 1])

        def proj_dr(ps, wh_t, wl_t):
            """ps += (xh+xl) @ (wh+wl) via 3-term residual DoubleRow.

            wh_t/wl_t are [128, KT, DH] per-target weight views."""
            terms = [(wh_t, xth), (wh_t, xtl), (wl_t, xth)]
            for ti, (w_t, x_sb) in enumerate(terms):
                for tp in range(KP):
                    nc.tensor.matmul(
                        ps, w_t[:, 2 * tp:2 * tp + 2, :],
                        x_sb[:, 2 * tp:2 * tp + 2, :],
                        start=(ti == 0 and tp == 0),
                        stop=(ti == 2 and tp == KP - 1),
                        perf_mode=DR)

        def rope(dst, ps, bias_col):
            """dst[:, ssl] = rope(ps*PSC + bias)."""
            raw = sb.tile([128, SC], F16, tag="qraw", name="qraw", bufs=4)
            nc.scalar.activation(raw, ps, AF.Identity, bias=bias_col,
                                 scale=PSC)
            rot = psum.tile([128, SC], F32, tag="score", name="rotps",
                            bufs=3)
            nc.tensor.matmul(rot, swap_sb, raw, start=True, stop=True)
            tmp = sb.tile([128, SC], F16, tag="ropetmp", name="ropetmp",
                          bufs=2)
            nc.vector.tensor_mul(dst[:, ssl], raw, cost_ap(ssl))
            nc.vector.tensor_mul(tmp, rot, sint_ap(ssl))
            nc.gpsimd.tensor_add(dst[:, ssl], dst[:, ssl], tmp)

        # v first: its ACT-copy + PE-transpose chain overlaps the q matmuls
        ps = psum.tile([128, SC], F32, tag="av", name="proj", bufs=2)
        proj_dr(ps, wkvh_sb[:, 1], wkvl_sb[:, 1])
        vt = sb.tile([128, SC], F16, tag="vt", name="vt", bufs=1)
        nc.scalar.activation(vt, ps, AF.Identity, bias=kvb_sb[:, 1:2],
                             scale=PSC)

        # k
        ps = psum.tile([128, SC], F32, tag="op", name="proj", bufs=2)
        proj_dr(ps, wkvh_sb[:, 0], wkvl_sb[:, 0])
        rope(k_rope, ps, kvb_sb[:, 0:1])

        for h in range(HPG):
            ps = psum.tile([128, SC], F32, tag=["av", "op"][h % 2],
                           name="proj", bufs=2)
            proj_dr(ps, wqh_sb[:, h], wql_sb[:, h])
            rope(q_rope[h], ps, qb_sb[:, h:h + 1])
            if h == 0:
                for j in range(SC // 128):
                    tp = psum.tile([128, 128], F16, tag="score",
                                   name="tpose", bufs=3)
                    nc.tensor.transpose(tp, vt[:, j * 128:(j + 1) * 128],
                                        ident_sb)
                    nc.scalar.activation(v_nat[:, sc * (SC // 128) + j, :],
                                         tp, AF.Copy)

    # ====== phases 2+3: causal attention (qc outer, head inner) with the
    # o-projection for q-chunk qc-1 interleaved into qc's attention ======
    out_pr = out_p.rearrange("(sb p) n -> p sb n", p=128)

    # wo reuses the (dead after phase 1) wq slots: same tag, same size.
    woh_sb = consts.tile([128, HPG, D], F8, tag="wqh", name="woh")
    wol_sb = consts.tile([128, HPG, D], F8, tag="wql", name="wol")
    nc.sync.dma_start(woh_sb, woh)
    nc.sync.dma_start(wol_sb, wol)

    opq = []

    def oproj_group(qc, dc, pair, last_qc):
        """One (dc, pair) slice of q-chunk qc's o-projection: 2 op tiles."""
        dsl = slice(dc * 512, (dc + 1) * 512)
        osb = sb.tile([128, 2, 512], F16, tag="osb", name="osb",
                      bufs=2)
        for j in range(2):
            sblk = qc * 4 + pair * 2 + j
            ssl2 = slice(sblk * 128, (sblk + 1) * 128)
            op = psum.tile([128, 512], F32, tag="op",
                           name="oproj", bufs=2)
            terms = [(attn8h, woh_sb), (attn8h, wol_sb),
                     (attn8l, woh_sb)]
            # cp-outer: heads 0-1 terms first so the final q-chunk's
            # o-projection starts before heads 2-3 attn8 lands
            for ci, cp in enumerate((0, 2)):
                for ti, (a8, w8) in enumerate(terms):
                    nc.tensor.matmul(
                        op, a8[:, cp:cp + 2, ssl2],
                        w8[:, cp:cp + 2, dsl],
                        start=(ci == 0 and ti == 0),
                        stop=(ci == 1 and ti == 2),
                        perf_mode=DR)
            if (sblk + dc) % 4 == 0:
                nc.scalar.activation(osb[:, j, :], op, AF.Identity,
                                     scale=OSC)
            else:
                nc.vector.tensor_scalar_mul(osb[:, j, :], op, OSC)
        if last_qc:
            for j in range(2):
                sblk = qc * 4 + pair * 2 + j
                q = [nc.sync, nc.scalar][(dc * 2 + pair + j) % 2]
                q.dma_start(out_pr[:, sblk:sblk + 1, dsl],
                            osb[:, j:j + 1, :])
        else:
            nc.sync.dma_start(
                out_pr[:, qc * 4 + pair * 2:qc * 4 + pair * 2 + 2, dsl],
                osb)

    # Flat 2-deep pipeline over ALL (qc, h, kj) regions.
    units = [(h, qc) for qc in range(NQC) for h in range(HPG)]
    seq = []
    for ui, (h, qc) in enumerate(units):
        for kj in range(4 * qc + 4):
            seq.append((ui, kj))
    ustate = {}

    def emit_scores(ui, kj):
        h, qc = units[ui]
        st = max(0, kj * 128 - qc * QC)
        width = QC - st
        sp = psum.tile([128, QC], F32, tag="score", name="score", bufs=3)
        nc.tensor.matmul(
            sp[:, 0:width],
            k_rope[:, kj * 128:(kj + 1) * 128],
            q_rope[h][:, qc * QC + st:(qc + 1) * QC],
            start=True, stop=True)
        if kj >= 4 * qc:  # region starts at the diagonal block
            nc.vector.tensor_add(sp[:, 0:128], sp[:, 0:128], mask_sb)
        et = sb.tile([128, QC], F16, tag="exp", name="exp", bufs=4)
        nc.scalar.activation(et[:, 0:width], sp[:, 0:width], AF.Exp)
        return et, st, width

    def emit_av(ui, kj, ready):
        h, qc = units[ui]
        et, st, width = ready
        if kj == 0:
            ustate[ui] = psum.tile([128, QC], F32, tag="av", name="av",
                                   bufs=2)
        att_ps = ustate[ui]
        row = 32 * (ui % 2)
        last = kj == 4 * qc + 3
        nc.tensor.matmul(
            att_ps[:, st:QC], v_nat[:, kj, :], et[:, 0:width],
            start=(kj == 0), stop=last, skip_group_check=True)
        nc.tensor.matmul(
            sums_ps[row:row + 1, st:QC], onesk_sb, et[:, 0:width],
            start=(kj == 0), stop=last, skip_group_check=True)
        if last:
            rec = sb.tile([1, QC], F32, tag="rec", name="rec", bufs=2)
            nc.vector.reciprocal(rec, sums_ps[row:row + 1, :])
            bcs = sb.tile([128, QC], F32, tag="bcs", name="bcs", bufs=2)
            t32 = sb.tile([128, QC], F16, tag="t32", name="t32", bufs=2)
            # t32 = SA * attn (fp16), then split into fp8 hi (ACT) + lo (DVE).
            # For the very last unit, run the chain in 128-col pieces so the
            # final o-projection (which consumes s-blocks in order) starts
            # ~2.5us earlier instead of waiting for the full-width chain.
            pieces = 4 if ui == len(units) - 1 else 1
            pw = QC // pieces
            for p in range(pieces):
                psl = slice(p * pw, (p + 1) * pw)
                qsl = slice(qc * QC + p * pw, qc * QC + (p + 1) * pw)
                nc.gpsimd.partition_broadcast(bcs[:, psl], rec[0:1, psl],
                                              channels=128)
                nc.vector.tensor_mul(t32[:, psl], att_ps[:, psl],
                                     bcs[:, psl])
                nc.scalar.activation(attn8h[:, h, qsl], t32[:, psl], AF.Copy)
                nc.vector.tensor_sub(attn8l[:, h, qsl], t32[:, psl],
                                     attn8h[:, h, qsl])
            del ustate[ui]
            if h == 3:
                opq.extend((qc, dc, pair) for dc in range(4)
                           for pair in range(2))

    LOOKAHEAD = 3
    ready = {}
    for i in range(min(LOOKAHEAD, len(seq))):
        ready[i] = emit_scores(*seq[i])
    for i in range(len(seq)):
        nxt = i + LOOKAHEAD
        if nxt < len(seq):
            ready[nxt] = emit_scores(*seq[nxt])
        emit_av(*seq[i], ready.pop(i))
        if i % 5 == 4 and opq and opq[0][0] < units[seq[i][0]][1]:
            oproj_group(*opq.pop(0), last_qc=False)
    while opq:
        item = opq.pop(0)
        oproj_group(*item, last_qc=item[0] == NQC - 1)

    psum.release()
    sb.release()
    consts.release()


def _host_tables():
    c4 = np.float32(1.0) / np.sqrt(np.sqrt(np.float32(DH)))
    inv_freq = (np.float32(1.0) / np.power(
        np.float32(10000.0),
        np.arange(0, DH, 2, dtype=np.float32) / np.float32(DH))).astype(np.float32)
    t = np.arange(S, dtype=np.float32)
    freqs = np.outer(t, inv_freq).astype(np.float32)          # [S, 64]
    emb = np.concatenate([freqs, freqs], axis=1)              # [S, 128]
    cost = (np.cos(emb).T * c4).astype(np.float32)            # [128, S]
    sint = (np.sin(emb).T * c4).astype(np.float32)
    sint[0:64] *= np.float32(-1.0)                            # rotate_half sign
    kq = np.arange(128, dtype=np.int64)
    mask = np.where(kq[None, :] >= kq[:, None], np.float32(0.0),
                    np.float32(-1e9)).astype(np.float32)      # [k,q]
    ident = np.eye(128, dtype=np.float32)
    # swap[i, j] = 1 iff j == (i+64) % 128; symmetric, so it works as lhsT.
    swap = np.zeros((128, 128), np.float32)
    swap[kq, (kq + 64) % 128] = np.float32(1.0)

    c16 = np.zeros((128, C16_W), np.float16)
    for sc in range(NSC):
        c16[:, sc * 2 * SC:sc * 2 * SC + SC] = cost[:, sc * SC:(sc + 1) * SC]
        c16[:, sc * 2 * SC + SC:(sc + 1) * 2 * SC] = \
            sint[:, sc * SC:(sc + 1) * SC]
    c16[:, C16_IDENT:C16_IDENT + 128] = ident
    c16[:, C16_SWAP:C16_SWAP + 128] = swap
    c16[:, C16_ONES] = np.float16(1.0 / SA)
    return c16


def _split8(a, scale):
    """Scaled hi/lo fp8 split: a*scale == hi + lo (to fp8^2 precision)."""
    sa = (a.astype(np.float32) * np.float32(scale))
    hi = sa.astype(NP_F8)
    lo = (sa - hi.astype(np.float32)).astype(NP_F8)
    return np.ascontiguousarray(hi), np.ascontiguousarray(lo)


def _pack_x(xT):
    """[D, S] -> [128, NSC, KT, SC] (p, sc, t, s); D = t*128+p, S = sc*SC+s."""
    v = xT.reshape(KT, 128, NSC, SC)
    return np.ascontiguousarray(v.transpose(1, 2, 0, 3))


def _pack_wq(wT):
    """[D, C] -> [128, HPG, KT, DH] (p, h, t, d); D = t*128+p, C = h*DH+d."""
    v = wT.reshape(KT, 128, HPG, DH)
    return np.ascontiguousarray(v.transpose(1, 2, 0, 3))


def _pack_wkv(wT):
    """[D, 2*DH] -> [128, 2, KT, DH] (p, kv, t, d)."""
    v = wT.reshape(KT, 128, 2, DH)
    return np.ascontiguousarray(v.transpose(1, 2, 0, 3))


def _pack_wo(wT):
    """[C, D] -> [128, HPG, D] (p, h, n); C = h*DH+p."""
    v = wT.reshape(HPG, 128, D)
    return np.ascontiguousarray(v.transpose(1, 0, 2))


def kernel(x, q_weight, q_bias, kv_weight, kv_bias, o_weight, o_bias):
    x = np.asarray(x, np.float32)
    q_weight = np.asarray(q_weight, np.float32)
    q_bias = np.asarray(q_bias, np.float32)
    kv_weight = np.asarray(kv_weight, np.float32)
    kv_bias = np.asarray(kv_bias, np.float32)
    o_weight = np.asarray(o_weight, np.float32)
    o_bias = np.asarray(o_bias, np.float32)

    if "nc" not in _NC_CACHE:
        _NC_CACHE["nc"] = build_nc()
    nc = _NC_CACHE["nc"]

    c16 = _host_tables()

    xs = []
    for b in range(B):
        hi, lo = _split8(x[b].T, SX)
        xs.append((_pack_x(hi), _pack_x(lo)))
    wkvh, wkvl = _split8(kv_weight.T, SW)
    wkvh, wkvl = _pack_wkv(wkvh), _pack_wkv(wkvl)

    in_maps = []
    for core in range(8):
        b, g = divmod(core, G)
        c32 = np.zeros((128, C32_W), np.float32)
        kq = np.arange(128, dtype=np.int64)
        c32[:, 0:128] = np.where(kq[None, :] >= kq[:, None], np.float32(0.0),
                                 np.float32(-1e9))
        c32[:, C32_QB:C32_QB + HPG] = \
            q_bias[g * C:(g + 1) * C].reshape(HPG, DH).T
        c32[:, C32_KVB:C32_KVB + 2] = kv_bias.reshape(2, DH).T

        wqh_, wql_ = _split8(q_weight[g * C:(g + 1) * C].T, SW)
        wqh_, wql_ = _pack_wq(wqh_), _pack_wq(wql_)
        woh_, wol_ = _split8(o_weight[:, g * C:(g + 1) * C].T, SW)
        woh_, wol_ = _pack_wo(woh_), _pack_wo(wol_)
        in_maps.append({
            "xh": xs[b][0],
            "xl": xs[b][1],
            "wqh": wqh_,
            "wql": wql_,
            "wkvh": wkvh,
            "wkvl": wkvl,
            "woh": woh_,
            "wol": wol_,
            "c16": c16,
            "c32": c32,
        })

    res = run_bass_kernel_spmd(nc, in_maps, core_ids=list(range(8)))

    out = np.zeros((B, S, D), np.float32)
    for core in range(8):
        out[core // G] += res.results[core]["out_p"].astype(np.float32)
    out += o_bias[None, None, :]
    return out


# revision 16
# speedup vs baseline: 1.0329x; 1.0329x over previous
"""Causal self-MQA kernel for Trainium2, sharded over 8 NeuronCores.

Problem: B=2, S=2048, D=2048, H=16 query heads, DH=128, single KV head,
GPT-NeoX RoPE, causal attention, fused q/kv/o projections.

Sharding: 8 cores = 2 batches x 4 head-groups (4 heads = 512 q-dims per
core). The tiny kv projection is replicated within a batch. Each core
computes a partial output [S, D] (its head-group's contribution through
the o-projection); the host sums the 4 partials per batch and adds
o_bias.

Datapath: fp16 activations end-to-end (PE fp16 = 1 col/cycle at any
width, so no f32r >=256 free-dim constraint and half the DMA bytes).
The two big weight GEMMs (q/kv projections, o-projection) run as
residual-fp8 DoubleRow matmuls: both operands are split hi+lo in
float8e4 with power-of-2 scaling (x*16, W*1024, attn*32), and the
product is computed as hi@hi + hi@lo + lo@hi at 0.5 cycles/col --
2x the fp16 rate with ~7-bit effective mantissa. The single shared
scale per term group means all three terms accumulate into one PSUM
bank; the descale folds into the (anyway needed) ACT/DVE copy out of
PSUM.

Layouts keep the feature dim on partitions so no activation transpose
is needed except the tiny V re-layout (32 PE transposes / core):
  qT[dh, s] = wqT.T @ xT          (DoubleRow pairs of d-tiles)
  rotate_half(q) = swap_matrix @ qT   (PE matmul; sign folded into sinT)
  scoresT[k, q] = k_ropeT(dh,k).T @ q_ropeT(dh,q)   (fp16, exact causal
    widths 512/384/256/128 -- no diagonal widening needed)
  softmax over k = PARTITION dim: no max-subtraction (scores < ~5),
    sums via ones-vector fp16 matmuls into a persistent 2-row PSUM tile
    (rows at partitions 0/32, alternating per unit, so the reciprocal
    read never serializes back-to-back units), reciprocal on DVE,
    partition-broadcast on GpSimd.
  attnT[dh, q] += v_nat(k,dh).T @ expT(k,q)   accumulated over k blocks
  attn stored directly as scaled fp8 hi/lo (32x) for the o-projection
  out_part[s_blk, d] = attn8 packs @ wo8 packs (DoubleRow), fp16 out
"""

import os
import sys

import numpy as np

for _p in ("/opt/trn_rl_repo", "/root/.axon_site/_ro/trn_rl_repo"):
    if os.path.isdir(_p) and _p not in sys.path:
        sys.path.insert(0, _p)

import ml_dtypes  # noqa: E402

import concourse.bass as bass  # noqa: E402,F401
import concourse.mybir as mybir  # noqa: E402
import concourse.tile as tile  # noqa: E402
from concourse import bacc  # noqa: E402
from concourse.bass_utils import run_bass_kernel_spmd  # noqa: E402

B, S, D = 2, 2048, 2048
H, DH = 16, 128
G = 4          # head groups (cores per batch)
HPG = 4        # heads per group
C = HPG * DH   # 512 output dims per group
SC = 256       # projection s-chunk width
NSC = S // SC  # 8
KT = D // 128  # 16 contraction tiles
KP = KT // 2   # 8 DoubleRow tile-pairs
QC = 512       # attention q-chunk width
NQC = S // QC  # 4
NSB = S // 128  # 16 s-blocks

SX = 16.0      # x fp8 scale
SW = 1024.0    # weight fp8 scale
SA = 32.0      # attn fp8 scale
PSC = 1.0 / (SX * SW)   # projection descale
OSC = 1.0 / (SA * SW)   # o-projection descale

F32 = mybir.dt.float32
F16 = mybir.dt.float16
F8 = mybir.dt.float8e4
AF = mybir.ActivationFunctionType
OP = mybir.AluOpType
DR = mybir.MatmulPerfMode.DoubleRow

NP_F8 = ml_dtypes.float8_e4m3

# packed fp16 consts layout: cost/sint interleaved per s-chunk, then
# ident / swap / onesk columns
C16_IDENT = NSC * 2 * SC            # 4096
C16_SWAP = C16_IDENT + 128          # 4224
C16_ONES = C16_SWAP + 128           # 4352
C16_W = C16_ONES + 1                # 4353
# packed f32 consts: mask, then qb (HPG cols), then kvb (2 cols)
C32_QB = 128
C32_KVB = C32_QB + HPG              # 132
C32_W = C32_KVB + 2                 # 134

_NC_CACHE = {}


def build_nc():
    nc = bacc.Bacc("TRN2", target_bir_lowering=False, debug=False)

    # all inputs pre-packed on the host into exact SBUF consumption layout
    # so every DMA moves >=2KB contiguous runs (no <512B descriptor penalty)
    xh = nc.dram_tensor("xh", [128, NSC, KT, SC], F8, kind="ExternalInput").ap()
    xl = nc.dram_tensor("xl", [128, NSC, KT, SC], F8, kind="ExternalInput").ap()
    wqh = nc.dram_tensor("wqh", [128, HPG, KT, DH], F8, kind="ExternalInput").ap()
    wql = nc.dram_tensor("wql", [128, HPG, KT, DH], F8, kind="ExternalInput").ap()
    wkvh = nc.dram_tensor("wkvh", [128, 2, KT, DH], F8, kind="ExternalInput").ap()
    wkvl = nc.dram_tensor("wkvl", [128, 2, KT, DH], F8, kind="ExternalInput").ap()
    woh = nc.dram_tensor("woh", [128, HPG, D], F8, kind="ExternalInput").ap()
    wol = nc.dram_tensor("wol", [128, HPG, D], F8, kind="ExternalInput").ap()
    c16 = nc.dram_tensor("c16", [128, C16_W], F16, kind="ExternalInput").ap()
    c32 = nc.dram_tensor("c32", [128, C32_W], F32, kind="ExternalInput").ap()
    out_p = nc.dram_tensor("out_p", [S, D], F16, kind="ExternalOutput").ap()

    with tile.TileContext(nc) as tc:
        _body(nc, tc, xh, xl, wqh, wql, wkvh, wkvl, woh, wol, c16, c32,
              out_p)
    nc.compile()
    return nc


def _body(nc, tc, xh, xl, wqh, wql, wkvh, wkvl, woh, wol, c16, c32, out_p):
    consts = tc.alloc_tile_pool(name="consts", bufs=1)
    sb = tc.alloc_tile_pool(name="sb", bufs=2)
    psum = tc.alloc_tile_pool(name="psum", bufs=1, space="PSUM")

    # ---- packed constants ----
    c16_sb = consts.tile([128, C16_W], F16, tag="c16", name="c16")
    c32_sb = consts.tile([128, C32_W], F32, tag="c32", name="c32")

    def cost_ap(ssl):  # ssl must be an s-chunk slice
        sc = ssl.start // SC
        off = sc * 2 * SC
        return c16_sb[:, off + (ssl.start - sc * SC):off + (ssl.stop - sc * SC)]

    def sint_ap(ssl):
        sc = ssl.start // SC
        off = sc * 2 * SC + SC
        return c16_sb[:, off + (ssl.start - sc * SC):off + (ssl.stop - sc * SC)]

    ident_sb = c16_sb[:, C16_IDENT:C16_IDENT + 128]
    swap_sb = c16_sb[:, C16_SWAP:C16_SWAP + 128]
    onesk_sb = c16_sb[:, C16_ONES:C16_ONES + 1]
    mask_sb = c32_sb[:, 0:128]
    qb_sb = c32_sb[:, C32_QB:C32_QB + HPG]
    kvb_sb = c32_sb[:, C32_KVB:C32_KVB + 2]

    # ---- weights (fp8 hi/lo) ----
    wqh_sb = consts.tile([128, HPG, KT, DH], F8, tag="wqh", name="wqh")
    wql_sb = consts.tile([128, HPG, KT, DH], F8, tag="wql", name="wql")
    wkvh_sb = consts.tile([128, 2, KT, DH], F8, tag="wkvh", name="wkvh")
    wkvl_sb = consts.tile([128, 2, KT, DH], F8, tag="wkvl", name="wkvl")

    # ---- persistent activations ----
    q_rope = [consts.tile([DH, S], F16, tag=f"qrope{h}", name=f"qrope{h}")
              for h in range(HPG)]
    k_rope = consts.tile([DH, S], F16, tag="krope", name="krope")
    v_nat = consts.tile([128, NSB, DH], F16, tag="vnat", name="vnat")
    attn8h = consts.tile([128, HPG, S], F8, tag="attn8h", name="attn8h")
    attn8l = consts.tile([128, HPG, S], F8, tag="attn8l", name="attn8l")

    # persistent 2-row softmax-sum PSUM tile (rows at partitions 0 and 32
    # so tile_position stays 32-aligned); alternating rows give the
    # reciprocal read two units of slack before the row is reused.
    sums_ps = psum.tile([64, QC], F32, tag="sums", name="sums")

    # ================= phase 1: q/kv projections + RoPE =================
    def alloc_x():
        return (sb.tile([128, KT, SC], F8, tag="xth", name="xth", bufs=3),
                sb.tile([128, KT, SC], F8, tag="xtl", name="xtl", bufs=3))

    xt_next = alloc_x()
    for sc in range(NSC):
        ssl = slice(sc * SC, (sc + 1) * SC)
        xth, xtl = xt_next
        if sc == 0:
            # startup: every input DMA on the (compute-free) SP queue, in
            # exact first-consumption order with small leading pieces. DMA
            # issues on the ACT/Pool queues would delay their compute.
            nc.sync.dma_start(wkvh_sb[:, 1, 0:2, :], wkvh[:, 1, 0:2, :])
            nc.sync.dma_start(xth[:, 0:8, :], xh[:, 0, 0:8, :])
            nc.sync.dma_start(wkvh_sb[:, 1, 2:KT, :], wkvh[:, 1, 2:KT, :])
            nc.sync.dma_start(xth[:, 8:16, :], xh[:, 0, 8:16, :])
            nc.sync.dma_start(xtl, xl[:, 0])
            nc.sync.dma_start(wkvl_sb[:, 1], wkvl[:, 1])    # v lo
            nc.sync.dma_start(wkvh_sb[:, 0], wkvh[:, 0])    # k hi
            nc.sync.dma_start(c32_sb, c32)
            nc.sync.dma_start(wkvl_sb[:, 0], wkvl[:, 0])    # k lo
            nc.sync.dma_start(c16_sb[:, C16_IDENT:C16_W],
                              c16[:, C16_IDENT:C16_W])
            nc.sync.dma_start(wqh_sb[:, 0], wqh[:, 0])
            nc.sync.dma_start(wql_sb[:, 0], wql[:, 0])
            nc.sync.dma_start(c16_sb[:, 0:1024], c16[:, 0:1024])
            nc.sync.dma_start(wqh_sb[:, 1], wqh[:, 1])
            nc.sync.dma_start(wql_sb[:, 1], wql[:, 1])
            xt_next = alloc_x()
            nc.sync.dma_start(xt_next[0], xh[:, 1])
            nc.sync.dma_start(wqh_sb[:, 2], wqh[:, 2])
            nc.sync.dma_start(wql_sb[:, 2], wql[:, 2])
            nc.sync.dma_start(xt_next[1], xl[:, 1])
            nc.sync.dma_start(wqh_sb[:, 3], wqh[:, 3])
            nc.sync.dma_start(wql_sb[:, 3], wql[:, 3])
            nc.sync.dma_start(c16_sb[:, 1024:C16_IDENT],
                              c16[:, 1024:C16_IDENT])
        elif sc < NSC - 1:
            xt_next = alloc_x()
            nc.sync.dma_start(xt_next[0], xh[:, sc + 1])
            nc.sync.dma_start(xt_next[1], xl[:, sc + 1])

        def proj_dr(ps, wh_t, wl_t):
            """ps += (xh+xl) @ (wh+wl) via 3-term residual DoubleRow.

            wh_t/wl_t are [128, KT, DH] per-target weight views."""
            terms = [(wh_t, xth), (wh_t, xtl), (wl_t, xth)]
            for ti, (w_t, x_sb) in enumerate(terms):
                for tp in range(KP):
                    nc.tensor.matmul(
                        ps, w_t[:, 2 * tp:2 * tp + 2, :],
                        x_sb[:, 2 * tp:2 * tp + 2, :],
                        start=(ti == 0 and tp == 0),
                        stop=(ti == 2 and tp == KP - 1),
                        perf_mode=DR)

        def rope(dst, ps, bias_col):
            """dst[:, ssl] = rope(ps*PSC + bias)."""
            raw = sb.tile([128, SC], F16, tag="qraw", name="qraw", bufs=4)
            nc.scalar.activation(raw, ps, AF.Identity, bias=bias_col,
                                 scale=PSC)
            rot = psum.tile([128, SC], F32, tag="score", name="rotps",
                            bufs=3)
            nc.tensor.matmul(rot, swap_sb, raw, start=True, stop=True)
            tmp = sb.tile([128, SC], F16, tag="ropetmp", name="ropetmp",
                          bufs=2)
            nc.vector.tensor_mul(dst[:, ssl], raw, cost_ap(ssl))
            nc.vector.tensor_mul(tmp, rot, sint_ap(ssl))
            nc.gpsimd.tensor_add(dst[:, ssl], dst[:, ssl], tmp)

        # v first: its ACT-copy + PE-transpose chain overlaps the q matmuls
        ps = psum.tile([128, SC], F32, tag="av", name="proj", bufs=2)
        proj_dr(ps, wkvh_sb[:, 1], wkvl_sb[:, 1])
        vt = sb.tile([128, SC], F16, tag="vt", name="vt", bufs=1)
        nc.scalar.activation(vt, ps, AF.Identity, bias=kvb_sb[:, 1:2],
                             scale=PSC)

        # k
        ps = psum.tile([128, SC], F32, tag="op", name="proj", bufs=2)
        proj_dr(ps, wkvh_sb[:, 0], wkvl_sb[:, 0])
        rope(k_rope, ps, kvb_sb[:, 0:1])

        for h in range(HPG):
            ps = psum.tile([128, SC], F32, tag=["av", "op"][h % 2],
                           name="proj", bufs=2)
            proj_dr(ps, wqh_sb[:, h], wql_sb[:, h])
            rope(q_rope[h], ps, qb_sb[:, h:h + 1])
            if h == 0:
                for j in range(SC // 128):
                    tp = psum.tile([128, 128], F16, tag="score",
                                   name="tpose", bufs=3)
                    nc.tensor.transpose(tp, vt[:, j * 128:(j + 1) * 128],
                                        ident_sb)
                    nc.scalar.activation(v_nat[:, sc * (SC // 128) + j, :],
                                         tp, AF.Copy)

    # ====== phases 2+3: causal attention (qc outer, head inner) with the
    # o-projection for q-chunk qc-1 interleaved into qc's attention ======
    out_pr = out_p.rearrange("(sb p) n -> p sb n", p=128)

    # wo reuses the (dead after phase 1) wq slots: same tag, same size.
    # Allocated late (see driver below): the tag-reuse WAR plus the in-order
    # SP queue would otherwise deadlock against the phase-1 weight DMAs.
    wo_sb = {}

    opq = []

    def oproj_group(qc, dc, pair, last_qc):
        """One (dc, pair) slice of q-chunk qc's o-projection: 2 op tiles."""
        dsl = slice(dc * 512, (dc + 1) * 512)
        osb = sb.tile([128, 2, 512], F16, tag="osb", name="osb",
                      bufs=2)
        for j in range(2):
            sblk = qc * 4 + pair * 2 + j
            ssl2 = slice(sblk * 128, (sblk + 1) * 128)
            op = psum.tile([128, 512], F32, tag="op",
                           name="oproj", bufs=2)
            terms = [(attn8h, wo_sb["h"]), (attn8h, wo_sb["l"]),
                     (attn8l, wo_sb["h"])]
            # cp-outer: heads 0-1 terms first so the final q-chunk's
            # o-projection starts before heads 2-3 attn8 lands
            for ci, cp in enumerate((0, 2)):
                for ti, (a8, w8) in enumerate(terms):
                    nc.tensor.matmul(
                        op, a8[:, cp:cp + 2, ssl2],
                        w8[:, cp:cp + 2, dsl],
                        start=(ci == 0 and ti == 0),
                        stop=(ci == 1 and ti == 2),
                        perf_mode=DR)
            if (sblk + dc) % 4 == 0:
                nc.scalar.activation(osb[:, j, :], op, AF.Identity,
                                     scale=OSC)
            else:
                nc.vector.tensor_scalar_mul(osb[:, j, :], op, OSC)
        if last_qc:
            for j in range(2):
                sblk = qc * 4 + pair * 2 + j
                q = [nc.sync, nc.scalar][(dc * 2 + pair + j) % 2]
                q.dma_start(out_pr[:, sblk:sblk + 1, dsl],
                            osb[:, j:j + 1, :])
        else:
            nc.sync.dma_start(
                out_pr[:, qc * 4 + pair * 2:qc * 4 + pair * 2 + 2, dsl],
                osb)

    # Flat 2-deep pipeline over ALL (qc, h, kj) regions.
    units = [(h, qc) for qc in range(NQC) for h in range(HPG)]
    seq = []
    for ui, (h, qc) in enumerate(units):
        for kj in range(4 * qc + 4):
            seq.append((ui, kj))
    ustate = {}

    def emit_scores(ui, kj):
        h, qc = units[ui]
        st = max(0, kj * 128 - qc * QC)
        width = QC - st
        sp = psum.tile([128, QC], F32, tag="score", name="score", bufs=3)
        nc.tensor.matmul(
            sp[:, 0:width],
            k_rope[:, kj * 128:(kj + 1) * 128],
            q_rope[h][:, qc * QC + st:(qc + 1) * QC],
            start=True, stop=True)
        if kj >= 4 * qc:  # region starts at the diagonal block
            nc.vector.tensor_add(sp[:, 0:128], sp[:, 0:128], mask_sb)
        et = sb.tile([128, QC], F16, tag="exp", name="exp", bufs=4)
        nc.scalar.activation(et[:, 0:width], sp[:, 0:width], AF.Exp)
        return et, st, width

    def emit_av(ui, kj, ready):
        h, qc = units[ui]
        et, st, width = ready
        if kj == 0:
            ustate[ui] = psum.tile([128, QC], F32, tag="av", name="av",
                                   bufs=2)
        att_ps = ustate[ui]
        row = 32 * (ui % 2)
        last = kj == 4 * qc + 3
        nc.tensor.matmul(
            att_ps[:, st:QC], v_nat[:, kj, :], et[:, 0:width],
            start=(kj == 0), stop=last, skip_group_check=True)
        nc.tensor.matmul(
            sums_ps[row:row + 1, st:QC], onesk_sb, et[:, 0:width],
            start=(kj == 0), stop=last, skip_group_check=True)
        if last:
            rec = sb.tile([1, QC], F32, tag="rec", name="rec", bufs=2)
            nc.vector.reciprocal(rec, sums_ps[row:row + 1, :])
            bcs = sb.tile([128, QC], F32, tag="bcs", name="bcs", bufs=2)
            t32 = sb.tile([128, QC], F16, tag="t32", name="t32", bufs=2)
            # t32 = SA * attn (fp16), then split into fp8 hi (ACT) + lo (DVE).
            # For the very last unit, run the chain in 128-col pieces so the
            # final o-projection (which consumes s-blocks in order) starts
            # ~2.5us earlier instead of waiting for the full-width chain.
            pieces = 4 if ui == len(units) - 1 else 1
            pw = QC // pieces
            for p in range(pieces):
                psl = slice(p * pw, (p + 1) * pw)
                qsl = slice(qc * QC + p * pw, qc * QC + (p + 1) * pw)
                nc.gpsimd.partition_broadcast(bcs[:, psl], rec[0:1, psl],
                                              channels=128)
                nc.vector.tensor_mul(t32[:, psl], att_ps[:, psl],
                                     bcs[:, psl])
                nc.scalar.activation(attn8h[:, h, qsl], t32[:, psl], AF.Copy)
                nc.vector.tensor_sub(attn8l[:, h, qsl], t32[:, psl],
                                     attn8h[:, h, qsl])
            del ustate[ui]
            if h == 3:
                opq.extend((qc, dc, pair) for dc in range(4)
                           for pair in range(2))

    LOOKAHEAD = 3
    ready = {}
    for i in range(min(LOOKAHEAD, len(seq))):
        ready[i] = emit_scores(*seq[i])
    for i in range(len(seq)):
        nxt = i + LOOKAHEAD
        if nxt < len(seq):
            ready[nxt] = emit_scores(*seq[nxt])
        emit_av(*seq[i], ready.pop(i))
        if i % 5 == 4 and opq and opq[0][0] < units[seq[i][0]][1]:
            oproj_group(*opq.pop(0), last_qc=False)
    while opq:
        item = opq.pop(0)
        oproj_group(*item, last_qc=item[0] == NQC - 1)

    psum.release()
    sb.release()
    consts.release()


def _host_tables():
    c4 = np.float32(1.0) / np.sqrt(np.sqrt(np.float32(DH)))
    inv_freq = (np.float32(1.0) / np.power(
        np.float32(10000.0),
        np.arange(0, DH, 2, dtype=np.float32) / np.float32(DH))).astype(np.float32)
    t = np.arange(S, dtype=np.float32)
    freqs = np.outer(t, inv_freq).astype(np.float32)          # [S, 64]
    emb = np.concatenate([freqs, freqs], axis=1)              # [S, 128]
    cost = (np.cos(emb).T * c4).astype(np.float32)            # [128, S]
    sint = (np.sin(emb).T * c4).astype(np.float32)
    sint[0:64] *= np.float32(-1.0)                            # rotate_half sign
    kq = np.arange(128, dtype=np.int64)
    mask = np.where(kq[None, :] >= kq[:, None], np.float32(0.0),
                    np.float32(-1e9)).astype(np.float32)      # [k,q]
    ident = np.eye(128, dtype=np.float32)
    # swap[i, j] = 1 iff j == (i+64) % 128; symmetric, so it works as lhsT.
    swap = np.zeros((128, 128), np.float32)
    swap[kq, (kq + 64) % 128] = np.float32(1.0)

    c16 = np.zeros((128, C16_W), np.float16)
    for sc in range(NSC):
        c16[:, sc * 2 * SC:sc * 2 * SC + SC] = cost[:, sc * SC:(sc + 1) * SC]
        c16[:, sc * 2 * SC + SC:(sc + 1) * 2 * SC] = \
            sint[:, sc * SC:(sc + 1) * SC]
    c16[:, C16_IDENT:C16_IDENT + 128] = ident
    c16[:, C16_SWAP:C16_SWAP + 128] = swap
    c16[:, C16_ONES] = np.float16(1.0 / SA)
    return c16


def _split8(a, scale):
    """Scaled hi/lo fp8 split: a*scale == hi + lo (to fp8^2 precision)."""
    sa = (a.astype(np.float32) * np.float32(scale))
    hi = sa.astype(NP_F8)
    lo = (sa - hi.astype(np.float32)).astype(NP_F8)
    return np.ascontiguousarray(hi), np.ascontiguousarray(lo)


def _pack_x(xT):
    """[D, S] -> [128, NSC, KT, SC] (p, sc, t, s); D = t*128+p, S = sc*SC+s."""
    v = xT.reshape(KT, 128, NSC, SC)
    return np.ascontiguousarray(v.transpose(1, 2, 0, 3))


def _pack_wq(wT):
    """[D, C] -> [128, HPG, KT, DH] (p, h, t, d); D = t*128+p, C = h*DH+d."""
    v = wT.reshape(KT, 128, HPG, DH)
    return np.ascontiguousarray(v.transpose(1, 2, 0, 3))


def _pack_wkv(wT):
    """[D, 2*DH] -> [128, 2, KT, DH] (p, kv, t, d)."""
    v = wT.reshape(KT, 128, 2, DH)
    return np.ascontiguousarray(v.transpose(1, 2, 0, 3))


def _pack_wo(wT):
    """[C, D] -> [128, HPG, D] (p, h, n); C = h*DH+p."""
    v = wT.reshape(HPG, 128, D)
    return np.ascontiguousarray(v.transpose(1, 0, 2))


def kernel(x, q_weight, q_bias, kv_weight, kv_bias, o_weight, o_bias):
    x = np.asarray(x, np.float32)
    q_weight = np.asarray(q_weight, np.float32)
    q_bias = np.asarray(q_bias, np.float32)
    kv_weight = np.asarray(kv_weight, np.float32)
    kv_bias = np.asarray(kv_bias, np.float32)
    o_weight = np.asarray(o_weight, np.float32)
    o_bias = np.asarray(o_bias, np.float32)

    if "nc" not in _NC_CACHE:
        _NC_CACHE["nc"] = build_nc()
    nc = _NC_CACHE["nc"]

    c16 = _host_tables()

    xs = []
    for b in range(B):
        hi, lo = _split8(x[b].T, SX)
        xs.append((_pack_x(hi), _pack_x(lo)))
    wkvh, wkvl = _split8(kv_weight.T, SW)
    wkvh, wkvl = _pack_wkv(wkvh), _pack_wkv(wkvl)

    in_maps = []
    for core in range(8):
        b, g = divmod(core, G)
        c32 = np.zeros((128, C32_W), np.float32)
        kq = np.arange(128, dtype=np.int64)
        c32[:, 0:128] = np.where(kq[None, :] >= kq[:, None], np.float32(0.0),
                                 np.float32(-1e9))
        c32[:, C32_QB:C32_QB + HPG] = \
            q_bias[g * C:(g + 1) * C].reshape(HPG, DH).T
        c32[:, C32_KVB:C32_KVB + 2] = kv_bias.reshape(2, DH).T

        wqh_, wql_ = _split8(q_weight[g * C:(g + 1) * C].T, SW)
        wqh_, wql_ = _pack_wq(wqh_), _pack_wq(wql_)
        woh_, wol_ = _split8(o_weight[:, g * C:(g + 1) * C].T, SW)
        woh_, wol_ = _pack_wo(woh_), _pack_wo(wol_)
        in_maps.append({
            "xh": xs[b][0],
            "xl": xs[b][1],
            "wqh": wqh_,
            "wql": wql_,
            "wkvh": wkvh,
            "wkvl": wkvl,
            "woh": woh_,
            "wol": wol_,
            "c16": c16,
            "c32": c32,
        })

    res = run_bass_kernel_spmd(nc, in_maps, core_ids=list(range(8)))

    out = np.zeros((B, S, D), np.float32)
    for core in range(8):
        out[core // G] += res.results[core]["out_p"].astype(np.float32)
    out += o_bias[None, None, :]
    return out


# revision 17
# speedup vs baseline: 1.0605x; 1.0267x over previous
"""Causal self-MQA kernel for Trainium2, sharded over 8 NeuronCores.

Problem: B=2, S=2048, D=2048, H=16 query heads, DH=128, single KV head,
GPT-NeoX RoPE, causal attention, fused q/kv/o projections.

Sharding: 8 cores = 2 batches x 4 head-groups (4 heads = 512 q-dims per
core). The tiny kv projection is replicated within a batch. Each core
computes a partial output [S, D] (its head-group's contribution through
the o-projection); the host sums the 4 partials per batch and adds
o_bias.

Datapath: fp16 activations end-to-end (PE fp16 = 1 col/cycle at any
width, so no f32r >=256 free-dim constraint and half the DMA bytes).
The two big weight GEMMs (q/kv projections, o-projection) run as
residual-fp8 DoubleRow matmuls: both operands are split hi+lo in
float8e4 with power-of-2 scaling (x*16, W*1024, attn*32), and the
product is computed as hi@hi + hi@lo + lo@hi at 0.5 cycles/col --
2x the fp16 rate with ~7-bit effective mantissa. The single shared
scale per term group means all three terms accumulate into one PSUM
bank; the descale folds into the (anyway needed) ACT/DVE copy out of
PSUM.

Layouts keep the feature dim on partitions so no activation transpose
is needed except the tiny V re-layout (32 PE transposes / core):
  qT[dh, s] = wqT.T @ xT          (DoubleRow pairs of d-tiles)
  rotate_half(q) = swap_matrix @ qT   (PE matmul; sign folded into sinT)
  scoresT[k, q] = k_ropeT(dh,k).T @ q_ropeT(dh,q)   (fp16, exact causal
    widths 512/384/256/128 -- no diagonal widening needed)
  softmax over k = PARTITION dim: no max-subtraction (scores < ~5),
    sums via ones-vector fp16 matmuls into a persistent 2-row PSUM tile
    (rows at partitions 0/32, alternating per unit, so the reciprocal
    read never serializes back-to-back units), reciprocal on DVE,
    partition-broadcast on GpSimd.
  attnT[dh, q] += v_nat(k,dh).T @ expT(k,q)   accumulated over k blocks
  attn stored directly as scaled fp8 hi/lo (32x) for the o-projection
  out_part[s_blk, d] = attn8 packs @ wo8 packs (DoubleRow), fp16 out
"""

import os
import sys

import numpy as np

for _p in ("/opt/trn_rl_repo", "/root/.axon_site/_ro/trn_rl_repo"):
    if os.path.isdir(_p) and _p not in sys.path:
        sys.path.insert(0, _p)

import ml_dtypes  # noqa: E402

import concourse.bass as bass  # noqa: E402,F401
import concourse.mybir as mybir  # noqa: E402
import concourse.tile as tile  # noqa: E402
from concourse import bacc  # noqa: E402
from concourse.bass_utils import run_bass_kernel_spmd  # noqa: E402

B, S, D = 2, 2048, 2048
H, DH = 16, 128
G = 4          # head groups (cores per batch)
HPG = 4        # heads per group
C = HPG * DH   # 512 output dims per group
SC = 256       # projection s-chunk width
NSC = S // SC  # 8
KT = D // 128  # 16 contraction tiles
KP = KT // 2   # 8 DoubleRow tile-pairs
QC = 512       # attention q-chunk width
NQC = S // QC  # 4
NSB = S // 128  # 16 s-blocks

SX = 16.0      # x fp8 scale
SW = 1024.0    # weight fp8 scale
SA = 32.0      # attn fp8 scale
PSC = 1.0 / (SX * SW)   # projection descale
OSC = 1.0 / (SA * SW)   # o-projection descale

F32 = mybir.dt.float32
F16 = mybir.dt.float16
F8 = mybir.dt.float8e4
AF = mybir.ActivationFunctionType
OP = mybir.AluOpType
DR = mybir.MatmulPerfMode.DoubleRow

NP_F8 = ml_dtypes.float8_e4m3

# packed fp16 consts layout: cost/sint interleaved per s-chunk, then
# ident / swap / onesk columns
C16_IDENT = NSC * 2 * SC            # 4096
C16_SWAP = C16_IDENT + 128          # 4224
C16_ONES = C16_SWAP + 128           # 4352
C16_W = C16_ONES + 1                # 4353
# packed f32 consts: mask, then qb (HPG cols), then kvb (2 cols)
C32_QB = 128
C32_KVB = C32_QB + HPG              # 132
C32_W = C32_KVB + 2                 # 134

FP8_SUMS = True

_NC_CACHE = {}


def build_nc():
    nc = bacc.Bacc("TRN2", target_bir_lowering=False, debug=False)

    # all inputs pre-packed on the host into exact SBUF consumption layout
    # so every DMA moves >=2KB contiguous runs (no <512B descriptor penalty)
    xh = nc.dram_tensor("xh", [128, NSC, KT, SC], F8, kind="ExternalInput").ap()
    xl = nc.dram_tensor("xl", [128, NSC, KT, SC], F8, kind="ExternalInput").ap()
    wqh = nc.dram_tensor("wqh", [128, HPG, KT, DH], F8, kind="ExternalInput").ap()
    wql = nc.dram_tensor("wql", [128, HPG, KT, DH], F8, kind="ExternalInput").ap()
    wkvh = nc.dram_tensor("wkvh", [128, 2, KT, DH], F8, kind="ExternalInput").ap()
    wkvl = nc.dram_tensor("wkvl", [128, 2, KT, DH], F8, kind="ExternalInput").ap()
    woh = nc.dram_tensor("woh", [128, HPG, D], F8, kind="ExternalInput").ap()
    wol = nc.dram_tensor("wol", [128, HPG, D], F8, kind="ExternalInput").ap()
    c16 = nc.dram_tensor("c16", [128, C16_W], F16, kind="ExternalInput").ap()
    c32 = nc.dram_tensor("c32", [128, C32_W], F32, kind="ExternalInput").ap()
    out_p = nc.dram_tensor("out_p", [S, D], F16, kind="ExternalOutput").ap()

    with tile.TileContext(nc) as tc:
        _body(nc, tc, xh, xl, wqh, wql, wkvh, wkvl, woh, wol, c16, c32,
              out_p)
    nc.compile()
    return nc


def _body(nc, tc, xh, xl, wqh, wql, wkvh, wkvl, woh, wol, c16, c32, out_p):
    consts = tc.alloc_tile_pool(name="consts", bufs=1)
    sb = tc.alloc_tile_pool(name="sb", bufs=2)
    psum = tc.alloc_tile_pool(name="psum", bufs=1, space="PSUM")

    # ---- packed constants ----
    c16_sb = consts.tile([128, C16_W], F16, tag="c16", name="c16")
    c32_sb = consts.tile([128, C32_W], F32, tag="c32", name="c32")

    def cost_ap(ssl):  # ssl must be an s-chunk slice
        sc = ssl.start // SC
        off = sc * 2 * SC
        return c16_sb[:, off + (ssl.start - sc * SC):off + (ssl.stop - sc * SC)]

    def sint_ap(ssl):
        sc = ssl.start // SC
        off = sc * 2 * SC + SC
        return c16_sb[:, off + (ssl.start - sc * SC):off + (ssl.stop - sc * SC)]

    ident_sb = c16_sb[:, C16_IDENT:C16_IDENT + 128]
    swap_sb = c16_sb[:, C16_SWAP:C16_SWAP + 128]
    onesk_sb = c16_sb[:, C16_ONES:C16_ONES + 1]
    mask_sb = c32_sb[:, 0:128]
    qb_sb = c32_sb[:, C32_QB:C32_QB + HPG]
    kvb_sb = c32_sb[:, C32_KVB:C32_KVB + 2]

    # ---- weights (fp8 hi/lo) ----
    wqh_sb = consts.tile([128, HPG, KT, DH], F8, tag="wqh", name="wqh")
    wql_sb = consts.tile([128, HPG, KT, DH], F8, tag="wql", name="wql")
    wkvh_sb = consts.tile([128, 2, KT, DH], F8, tag="wkvh", name="wkvh")
    wkvl_sb = consts.tile([128, 2, KT, DH], F8, tag="wkvl", name="wkvl")

    # ---- persistent activations ----
    q_rope = [consts.tile([DH, S], F16, tag=f"qrope{h}", name=f"qrope{h}")
              for h in range(HPG)]
    k_rope = consts.tile([DH, S], F16, tag="krope", name="krope")
    v_nat = consts.tile([128, NSB, DH], F16, tag="vnat", name="vnat")
    attn8h = consts.tile([128, HPG, S], F8, tag="attn8h", name="attn8h")
    attn8l = consts.tile([128, HPG, S], F8, tag="attn8l", name="attn8l")

    ones8_sb = consts.tile([128, 2, 1], F8, tag="ones8", name="ones8")
    nc.vector.memset(ones8_sb, 1.0 / SA)

    # persistent 2-row softmax-sum PSUM tile (rows at partitions 0 and 32
    # so tile_position stays 32-aligned); alternating rows give the
    # reciprocal read two units of slack before the row is reused.
    sums_ps = psum.tile([64, QC], F32, tag="sums", name="sums")

    # ================= phase 1: q/kv projections + RoPE =================
    def alloc_x():
        return (sb.tile([128, KT, SC], F8, tag="xth", name="xth", bufs=3),
                sb.tile([128, KT, SC], F8, tag="xtl", name="xtl", bufs=3))

    xt_next = alloc_x()
    for sc in range(NSC):
        ssl = slice(sc * SC, (sc + 1) * SC)
        xth, xtl = xt_next
        if sc == 0:
            # startup: every input DMA on the (compute-free) SP queue, in
            # exact first-consumption order with small leading pieces. DMA
            # issues on the ACT/Pool queues would delay their compute.
            nc.sync.dma_start(wkvh_sb[:, 1, 0:2, :], wkvh[:, 1, 0:2, :])
            nc.sync.dma_start(xth[:, 0:8, :], xh[:, 0, 0:8, :])
            nc.sync.dma_start(wkvh_sb[:, 1, 2:KT, :], wkvh[:, 1, 2:KT, :])
            nc.sync.dma_start(xth[:, 8:16, :], xh[:, 0, 8:16, :])
            nc.sync.dma_start(xtl, xl[:, 0])
            nc.sync.dma_start(wkvl_sb[:, 1], wkvl[:, 1])    # v lo
            nc.sync.dma_start(wkvh_sb[:, 0], wkvh[:, 0])    # k hi
            nc.sync.dma_start(c32_sb, c32)
            nc.sync.dma_start(wkvl_sb[:, 0], wkvl[:, 0])    # k lo
            nc.sync.dma_start(c16_sb[:, C16_IDENT:C16_W],
                              c16[:, C16_IDENT:C16_W])
            nc.sync.dma_start(wqh_sb[:, 0], wqh[:, 0])
            nc.sync.dma_start(wql_sb[:, 0], wql[:, 0])
            nc.sync.dma_start(c16_sb[:, 0:1024], c16[:, 0:1024])
            nc.sync.dma_start(wqh_sb[:, 1], wqh[:, 1])
            nc.sync.dma_start(wql_sb[:, 1], wql[:, 1])
            xt_next = alloc_x()
            nc.sync.dma_start(xt_next[0], xh[:, 1])
            nc.sync.dma_start(wqh_sb[:, 2], wqh[:, 2])
            nc.sync.dma_start(wql_sb[:, 2], wql[:, 2])
            nc.sync.dma_start(xt_next[1], xl[:, 1])
            nc.sync.dma_start(wqh_sb[:, 3], wqh[:, 3])
            nc.sync.dma_start(wql_sb[:, 3], wql[:, 3])
            nc.sync.dma_start(c16_sb[:, 1024:C16_IDENT],
                              c16[:, 1024:C16_IDENT])
        elif sc < NSC - 1:
            xt_next = alloc_x()
            nc.sync.dma_start(xt_next[0], xh[:, sc + 1])
            nc.sync.dma_start(xt_next[1], xl[:, sc + 1])

        def proj_dr(ps, wh_t, wl_t):
            """ps += (xh+xl) @ (wh+wl) via 3-term residual DoubleRow.

            wh_t/wl_t are [128, KT, DH] per-target weight views."""
            terms = [(wh_t, xth), (wh_t, xtl), (wl_t, xth)]
            for ti, (w_t, x_sb) in enumerate(terms):
                for tp in range(KP):
                    nc.tensor.matmul(
                        ps, w_t[:, 2 * tp:2 * tp + 2, :],
                        x_sb[:, 2 * tp:2 * tp + 2, :],
                        start=(ti == 0 and tp == 0),
                        stop=(ti == 2 and tp == KP - 1),
                        perf_mode=DR)

        def rope(dst, ps, bias_col):
            """dst[:, ssl] = rope(ps*PSC + bias)."""
            raw = sb.tile([128, SC], F16, tag="qraw", name="qraw", bufs=4)
            nc.scalar.activation(raw, ps, AF.Identity, bias=bias_col,
                                 scale=PSC)
            rot = psum.tile([128, SC], F32, tag="score", name="rotps",
                            bufs=3)
            nc.tensor.matmul(rot, swap_sb, raw, start=True, stop=True)
            tmp = sb.tile([128, SC], F16, tag="ropetmp", name="ropetmp",
                          bufs=2)
            nc.vector.tensor_mul(dst[:, ssl], raw, cost_ap(ssl))
            nc.vector.tensor_mul(tmp, rot, sint_ap(ssl))
            nc.gpsimd.tensor_add(dst[:, ssl], dst[:, ssl], tmp)

        # v first: its ACT-copy + PE-transpose chain overlaps the q matmuls
        ps = psum.tile([128, SC], F32, tag="av", name="proj", bufs=2)
        proj_dr(ps, wkvh_sb[:, 1], wkvl_sb[:, 1])
        vt = sb.tile([128, SC], F16, tag="vt", name="vt", bufs=1)
        nc.scalar.activation(vt, ps, AF.Identity, bias=kvb_sb[:, 1:2],
                             scale=PSC)

        # k
        ps = psum.tile([128, SC], F32, tag="op", name="proj", bufs=2)
        proj_dr(ps, wkvh_sb[:, 0], wkvl_sb[:, 0])
        rope(k_rope, ps, kvb_sb[:, 0:1])

        for h in range(HPG):
            ps = psum.tile([128, SC], F32, tag=["av", "op"][h % 2],
                           name="proj", bufs=2)
            proj_dr(ps, wqh_sb[:, h], wql_sb[:, h])
            rope(q_rope[h], ps, qb_sb[:, h:h + 1])
            if h == 0:
                for j in range(SC // 128):
                    tp = psum.tile([128, 128], F16, tag="score",
                                   name="tpose", bufs=3)
                    nc.tensor.transpose(tp, vt[:, j * 128:(j + 1) * 128],
                                        ident_sb)
                    nc.scalar.activation(v_nat[:, sc * (SC // 128) + j, :],
                                         tp, AF.Copy)

    # ====== phases 2+3: causal attention (qc outer, head inner) with the
    # o-projection for q-chunk qc-1 interleaved into qc's attention ======
    out_pr = out_p.rearrange("(sb p) n -> p sb n", p=128)

    # wo reuses the (dead after phase 1) wq slots: same tag, same size.
    # Allocated late (see driver below): the tag-reuse WAR plus the in-order
    # SP queue would otherwise deadlock against the phase-1 weight DMAs.
    wo_sb = {}

    opq = []

    def oproj_group(qc, dc, pair, last_qc):
        """One (dc, pair) slice of q-chunk qc's o-projection: 2 op tiles."""
        dsl = slice(dc * 512, (dc + 1) * 512)
        osb = sb.tile([128, 2, 512], F16, tag="osb", name="osb",
                      bufs=2)
        for j in range(2):
            sblk = qc * 4 + pair * 2 + j
            ssl2 = slice(sblk * 128, (sblk + 1) * 128)
            op = psum.tile([128, 512], F32, tag="op",
                           name="oproj", bufs=2)
            terms = [(attn8h, wo_sb["h"]), (attn8h, wo_sb["l"]),
                     (attn8l, wo_sb["h"])]
            # cp-outer: heads 0-1 terms first so the final q-chunk's
            # o-projection starts before heads 2-3 attn8 lands
            for ci, cp in enumerate((0, 2)):
                for ti, (a8, w8) in enumerate(terms):
                    nc.tensor.matmul(
                        op, a8[:, cp:cp + 2, ssl2],
                        w8[:, cp:cp + 2, dsl],
                        start=(ci == 0 and ti == 0),
                        stop=(ci == 1 and ti == 2),
                        perf_mode=DR)
            if (sblk + dc) % 4 == 0:
                nc.scalar.activation(osb[:, j, :], op, AF.Identity,
                                     scale=OSC)
            else:
                nc.vector.tensor_scalar_mul(osb[:, j, :], op, OSC)
        if last_qc:
            for j in range(2):
                sblk = qc * 4 + pair * 2 + j
                q = [nc.sync, nc.scalar][(dc * 2 + pair + j) % 2]
                q.dma_start(out_pr[:, sblk:sblk + 1, dsl],
                            osb[:, j:j + 1, :])
        else:
            nc.sync.dma_start(
                out_pr[:, qc * 4 + pair * 2:qc * 4 + pair * 2 + 2, dsl],
                osb)

    # Flat 2-deep pipeline over ALL (qc, h, kj) regions.
    units = [(h, qc) for qc in range(NQC) for h in range(HPG)]
    seq = []
    for ui, (h, qc) in enumerate(units):
        for kj in range(4 * qc + 4):
            seq.append((ui, kj))
    ustate = {}

    def emit_scores(ui, kj):
        h, qc = units[ui]
        st = max(0, kj * 128 - qc * QC)
        width = QC - st
        sp = psum.tile([128, QC], F32, tag="score", name="score", bufs=3)
        nc.tensor.matmul(
            sp[:, 0:width],
            k_rope[:, kj * 128:(kj + 1) * 128],
            q_rope[h][:, qc * QC + st:(qc + 1) * QC],
            start=True, stop=True)
        if kj >= 4 * qc:  # region starts at the diagonal block
            nc.vector.tensor_add(sp[:, 0:128], sp[:, 0:128], mask_sb)
        et = sb.tile([128, QC], F16, tag="exp", name="exp", bufs=4)
        nc.scalar.activation(et[:, 0:width], sp[:, 0:width], AF.Exp)
        return et, st, width

    def emit_pair_sums(ui, pr, start, stop):
        e8, st0, w0 = pr
        row = 32 * (ui % 2)
        nc.tensor.matmul(
            sums_ps[row:row + 1, st0:QC], ones8_sb, e8[:, :, 0:w0],
            start=start, stop=stop, perf_mode=DR, skip_group_check=True)

    def emit_av(ui, kj, ready):
        h, qc = units[ui]
        et, st, width = ready
        if kj == 0:
            ustate[ui] = (psum.tile([128, QC], F32, tag="av", name="av",
                                    bufs=2), {"pairs": [], "done": 0})
        att_ps, pst = ustate[ui]
        row = 32 * (ui % 2)
        last = kj == 4 * qc + 3
        nc.tensor.matmul(
            att_ps[:, st:QC], v_nat[:, kj, :], et[:, 0:width],
            start=(kj == 0), stop=last, skip_group_check=True)
        if not FP8_SUMS:
            nc.tensor.matmul(
                sums_ps[row:row + 1, st:QC], onesk_sb, et[:, 0:width],
                start=(kj == 0), stop=last, skip_group_check=True)
        else:
            # fp8 DoubleRow softmax sums: two adjacent k-blocks pack into one
            # [128, 2, w] fp8 tile; the pair matmul runs at 0.25 cyc/col per
            # block. Casts go to the underused GpSimd (2/3) and DVE (1/3);
            # pair emission is delayed one pair for cast latency slack.
            cast = nc.vector if kj % 3 == 0 else nc.gpsimd
            if kj % 2 == 0:
                e8 = sb.tile([128, 2, QC], F8, tag="et8", name="et8", bufs=3)
                pst["cur"] = (e8, st, width)
                cast.tensor_copy(e8[:, 0, 0:width], et[:, 0:width])
            else:
                e8, st0, w0 = pst["cur"]
                pad = st - st0
                if pad:
                    nc.gpsimd.memset(e8[:, 1, 0:pad], 0.0)
                cast.tensor_copy(e8[:, 1, pad:pad + width], et[:, 0:width])
                pst["pairs"].append((e8, st0, w0))
                if len(pst["pairs"]) >= 2:
                    emit_pair_sums(ui, pst["pairs"].pop(0),
                                   start=(pst["done"] == 0), stop=False)
                    pst["done"] += 1
        if last:
            if FP8_SUMS:
                npairs = 2 * qc + 2
                while pst["pairs"]:
                    pr = pst["pairs"].pop(0)
                    emit_pair_sums(ui, pr, start=(pst["done"] == 0),
                                   stop=(pst["done"] == npairs - 1))
                    pst["done"] += 1
            rec = sb.tile([1, QC], F32, tag="rec", name="rec", bufs=2)
            nc.vector.reciprocal(rec, sums_ps[row:row + 1, :])
            att_ps = ustate[ui][0] if FP8_SUMS else att_ps
            bcs = sb.tile([128, QC], F32, tag="bcs", name="bcs", bufs=2)
            t32 = sb.tile([128, QC], F16, tag="t32", name="t32", bufs=2)
            # t32 = SA * attn (fp16), then split into fp8 hi (ACT) + lo (DVE).
            # For the very last unit, run the chain in 128-col pieces so the
            # final o-projection (which consumes s-blocks in order) starts
            # ~2.5us earlier instead of waiting for the full-width chain.
            pieces = 4 if ui == len(units) - 1 else 1
            pw = QC // pieces
            for p in range(pieces):
                psl = slice(p * pw, (p + 1) * pw)
                qsl = slice(qc * QC + p * pw, qc * QC + (p + 1) * pw)
                nc.gpsimd.partition_broadcast(bcs[:, psl], rec[0:1, psl],
                                              channels=128)
                nc.vector.tensor_mul(t32[:, psl], att_ps[:, psl],
                                     bcs[:, psl])
                nc.scalar.activation(attn8h[:, h, qsl], t32[:, psl], AF.Copy)
                nc.vector.tensor_sub(attn8l[:, h, qsl], t32[:, psl],
                                     attn8h[:, h, qsl])
            del ustate[ui]
            if h == 3:
                opq.extend((qc, dc, pair) for dc in range(4)
                           for pair in range(2))

    LOOKAHEAD = 3
    ready = {}
    for i in range(min(LOOKAHEAD, len(seq))):
        ready[i] = emit_scores(*seq[i])
    for i in range(len(seq)):
        nxt = i + LOOKAHEAD
        if nxt < len(seq):
            ready[nxt] = emit_scores(*seq[nxt])
        emit_av(*seq[i], ready.pop(i))
        if i % 5 == 4 and opq and opq[0][0] < units[seq[i][0]][1]:
            oproj_group(*opq.pop(0), last_qc=False)
    while opq:
        item = opq.pop(0)
        oproj_group(*item, last_qc=item[0] == NQC - 1)

    psum.release()
    sb.release()
    consts.release()


def _host_tables():
    c4 = np.float32(1.0) / np.sqrt(np.sqrt(np.float32(DH)))
    inv_freq = (np.float32(1.0) / np.power(
        np.float32(10000.0),
        np.arange(0, DH, 2, dtype=np.float32) / np.float32(DH))).astype(np.float32)
    t = np.arange(S, dtype=np.float32)
    freqs = np.outer(t, inv_freq).astype(np.float32)          # [S, 64]
    emb = np.concatenate([freqs, freqs], axis=1)              # [S, 128]
    cost = (np.cos(emb).T * c4).astype(np.float32)            # [128, S]
    sint = (np.sin(emb).T * c4).astype(np.float32)
    sint[0:64] *= np.float32(-1.0)                            # rotate_half sign
    kq = np.arange(128, dtype=np.int64)
    mask = np.where(kq[None, :] >= kq[:, None], np.float32(0.0),
                    np.float32(-1e9)).astype(np.float32)      # [k,q]
    ident = np.eye(128, dtype=np.float32)
    # swap[i, j] = 1 iff j == (i+64) % 128; symmetric, so it works as lhsT.
    swap = np.zeros((128, 128), np.float32)
    swap[kq, (kq + 64) % 128] = np.float32(1.0)

    c16 = np.zeros((128, C16_W), np.float16)
    for sc in range(NSC):
        c16[:, sc * 2 * SC:sc * 2 * SC + SC] = cost[:, sc * SC:(sc + 1) * SC]
        c16[:, sc * 2 * SC + SC:(sc + 1) * 2 * SC] = \
            sint[:, sc * SC:(sc + 1) * SC]
    c16[:, C16_IDENT:C16_IDENT + 128] = ident
    c16[:, C16_SWAP:C16_SWAP + 128] = swap
    c16[:, C16_ONES] = np.float16(1.0 / SA)
    return c16


def _split8(a, scale):
    """Scaled hi/lo fp8 split: a*scale == hi + lo (to fp8^2 precision)."""
    sa = (a.astype(np.float32) * np.float32(scale))
    hi = sa.astype(NP_F8)
    lo = (sa - hi.astype(np.float32)).astype(NP_F8)
    return np.ascontiguousarray(hi), np.ascontiguousarray(lo)


def _pack_x(xT):
    """[D, S] -> [128, NSC, KT, SC] (p, sc, t, s); D = t*128+p, S = sc*SC+s."""
    v = xT.reshape(KT, 128, NSC, SC)
    return np.ascontiguousarray(v.transpose(1, 2, 0, 3))


def _pack_wq(wT):
    """[D, C] -> [128, HPG, KT, DH] (p, h, t, d); D = t*128+p, C = h*DH+d."""
    v = wT.reshape(KT, 128, HPG, DH)
    return np.ascontiguousarray(v.transpose(1, 2, 0, 3))


def _pack_wkv(wT):
    """[D, 2*DH] -> [128, 2, KT, DH] (p, kv, t, d)."""
    v = wT.reshape(KT, 128, 2, DH)
    return np.ascontiguousarray(v.transpose(1, 2, 0, 3))


def _pack_wo(wT):
    """[C, D] -> [128, HPG, D] (p, h, n); C = h*DH+p."""
    v = wT.reshape(HPG, 128, D)
    return np.ascontiguousarray(v.transpose(1, 0, 2))


def kernel(x, q_weight, q_bias, kv_weight, kv_bias, o_weight, o_bias):
    x = np.asarray(x, np.float32)
    q_weight = np.asarray(q_weight, np.float32)
    q_bias = np.asarray(q_bias, np.float32)
    kv_weight = np.asarray(kv_weight, np.float32)
    kv_bias = np.asarray(kv_bias, np.float32)
    o_weight = np.asarray(o_weight, np.float32)
    o_bias = np.asarray(o_bias, np.float32)

    if "nc" not in _NC_CACHE:
        _NC_CACHE["nc"] = build_nc()
    nc = _NC_CACHE["nc"]

    c16 = _host_tables()

    xs = []
    for b in range(B):
        hi, lo = _split8(x[b].T, SX)
        xs.append((_pack_x(hi), _pack_x(lo)))
    wkvh, wkvl = _split8(kv_weight.T, SW)
    wkvh, wkvl = _pack_wkv(wkvh), _pack_wkv(wkvl)

    in_maps = []
    for core in range(8):
        b, g = divmod(core, G)
        c32 = np.zeros((128, C32_W), np.float32)
        kq = np.arange(128, dtype=np.int64)
        c32[:, 0:128] = np.where(kq[None, :] >= kq[:, None], np.float32(0.0),
                                 np.float32(-1e9))
        c32[:, C32_QB:C32_QB + HPG] = \
            q_bias[g * C:(g + 1) * C].reshape(HPG, DH).T
        c32[:, C32_KVB:C32_KVB + 2] = kv_bias.reshape(2, DH).T

        wqh_, wql_ = _split8(q_weight[g * C:(g + 1) * C].T, SW)
        wqh_, wql_ = _pack_wq(wqh_), _pack_wq(wql_)
        woh_, wol_ = _split8(o_weight[:, g * C:(g + 1) * C].T, SW)
        woh_, wol_ = _pack_wo(woh_), _pack_wo(wol_)
        in_maps.append({
            "xh": xs[b][0],
            "xl": xs[b][1],
            "wqh": wqh_,
            "wql": wql_,
            "wkvh": wkvh,
            "wkvl": wkvl,
            "woh": woh_,
            "wol": wol_,
            "c16": c16,
            "c32": c32,
        })

    res = run_bass_kernel_spmd(nc, in_maps, core_ids=list(range(8)))

    out = np.zeros((B, S, D), np.float32)
    for core in range(8):
        out[core // G] += res.results[core]["out_p"].astype(np.float32)
    out += o_bias[None, None, :]
    return out
